# revision 4
# baseline (speedup 1.0000x reference)
"""GateGAT on 8 Trainium2 NeuronCores.

Strategy: edges bucketed by destination node range (8 cores own contiguous
node ranges). Per-edge gathers via indirect DMA (128 rows/instr); segment
softmax+sum via one-hot matmuls into PSUM per 128-node destination block.
Cross-core: tiny AllReduce for the gate min/max, AllGather of z2/attn tables
and of the q/r edge-score tables.
"""
import numpy as np

import concourse.bass as bass
import concourse.bacc as bacc
import concourse.mybir as mybir
import concourse.tile as tile
from concourse.bass_utils import run_bass_kernel_spmd

P = 128
NCORES = 8
F32 = mybir.dt.float32
I32 = mybir.dt.int32

_cache = {}


def _build(NP, NPC, NDB, CPD, trace=False):
    key = (NP, NPC, NDB, CPD)
    if key in _cache:
        return _cache[key]
    NCHUNK = NDB * CPD
    NB = NP // P  # node blocks globally

    nc = bacc.Bacc("TRN2", target_bir_lowering=False, debug=False,
                   num_devices=NCORES)
    dt = nc.dram_tensor
    # inputs
    hT = dt("hT", [P, NP], F32, kind="ExternalInput")
    srcidx = dt("srcidx", [P, NCHUNK], I32, kind="ExternalInput")
    dstidx = dt("dstidx", [P, NCHUNK], I32, kind="ExternalInput")
    dstloc = dt("dstloc", [P, NCHUNK], F32, kind="ExternalInput")
    invbias = dt("invbias", [P, NCHUNK], F32, kind="ExternalInput")
    Wall = dt("Wall", [P, 288], F32, kind="ExternalInput")
    c2row = dt("c2row", [P, 8], F32, kind="ExternalInput")
    W3row = dt("W3row", [P, 8], F32, kind="ExternalInput")
    fc2aug = dt("fc2aug", [P, 132], F32, kind="ExternalInput")
    Wqr = dt("Wqr", [64, 4], F32, kind="ExternalInput")
    bprow = dt("bprow", [P, 2], F32, kind="ExternalInput")
    IOTA = dt("IOTA", [P, P], F32, kind="ExternalInput")
    IDN = dt("IDN", [P, P], F32, kind="ExternalInput")
    # outputs
    score_out = dt("score_out", [P, NCHUNK * 2], F32, kind="ExternalOutput")
    gate_out = dt("gate_out", [P, NCHUNK], F32, kind="ExternalOutput")
    # internal DRAM
    z1d = dt("z1d", [NP, 256], F32)
    t32d = dt("t32d", [NP, 32], F32)
    mm_in = dt("mm_in", [2, 1], F32)
    mm_out = dt("mm_out", [2, 1], F32, addr_space="Shared")
    z2in = dt("z2in", [NPC, 66], F32)
    z2all = dt("z2all", [NP, 66], F32, addr_space="Shared")
    qrin = dt("qrin", [NPC, 4], F32)
    qrall = dt("qrall", [NP, 4], F32, addr_space="Shared")

    RG = [list(range(NCORES))]

    def IOX(ap, axis=0):
        return bass.IndirectOffsetOnAxis(ap=ap, axis=axis)

    with tile.TileContext(nc) as tc:
        with (
            tc.tile_pool(name="slab", bufs=1) as slab,
            tc.tile_pool(name="work", bufs=3) as work,
            tc.tile_pool(name="gath", bufs=2) as gath,
            tc.tile_pool(name="psum", bufs=2, space="PSUM") as psum,
        ):
            # persistent consts
            Wall_t = slab.tile([P, 288], F32)
            nc.sync.dma_start(out=Wall_t[:], in_=Wall.ap())
            c2_t = slab.tile([P, 8], F32)
            nc.sync.dma_start(out=c2_t[:], in_=c2row.ap())
            w3_t = slab.tile([P, 8], F32)
            nc.sync.dma_start(out=w3_t[:], in_=W3row.ap())
            fc2_t = slab.tile([P, 2 * 66], F32)
            nc.sync.dma_start(out=fc2_t[:], in_=fc2aug.ap())
            wqr_t = slab.tile([64, 4], F32)
            nc.sync.dma_start(out=wqr_t[:], in_=Wqr.ap())
            bp_t = slab.tile([P, 2], F32)
            nc.sync.dma_start(out=bp_t[:], in_=bprow.ap())
            iota_t = slab.tile([P, P], F32)
            nc.sync.dma_start(out=iota_t[:], in_=IOTA.ap())
            idn_t = slab.tile([P, P], F32)
            nc.sync.dma_start(out=idn_t[:], in_=IDN.ap())
            dstloc_t = slab.tile([P, NCHUNK], F32)
            nc.sync.dma_start(out=dstloc_t[:], in_=dstloc.ap())
            srcidx_t = slab.tile([P, NCHUNK], I32)
            nc.sync.dma_start(out=srcidx_t[:], in_=srcidx.ap())
            dstidx_t = slab.tile([P, NCHUNK], I32)
            nc.sync.dma_start(out=dstidx_t[:], in_=dstidx.ap())
            # persistent slabs
            el_sl = slab.tile([P, NCHUNK * 8], F32)
            ex_sl = el_sl
            s_sl = slab.tile([P, NCHUNK], F32)
            gate_sl = s_sl
            hA_sl = slab.tile([P, NDB * 256], F32)
            hB_sl = slab.tile([P, NDB * 64], F32)
            score_sl = slab.tile([P, NCHUNK * 2], F32)

            def bc(apx, inner):
                """broadcast innermost: [...,(k)] -> [...,(k),inner]"""
                return bass.AP(apx.tensor, apx.offset, list(apx.ap) + [[0, inner]])

            # ---------------- P0: node tables ----------------
            for nb in range(NB):
                hs = work.tile([P, P], F32, tag="hchunk")
                nc.sync.dma_start(out=hs[:], in_=hT.ap()[:, nb * P:(nb + 1) * P])
                ps0 = psum.tile([P, 288], F32, tag="acc", space="PSUM")
                nc.tensor.matmul(out=ps0[:], lhsT=hs[:], rhs=Wall_t[:],
                                 start=True, stop=True)
                z1s = work.tile([P, 256], F32, tag="z1s")
                nc.vector.tensor_copy(out=z1s[:], in_=ps0[:, 0:256])
                t32s = work.tile([P, 32], F32, tag="t32s")
                nc.vector.tensor_copy(out=t32s[:], in_=ps0[:, 256:288])
                nc.sync.dma_start(out=z1d.ap()[nb * P:(nb + 1) * P, :], in_=z1s[:])
                nc.sync.dma_start(out=t32d.ap()[nb * P:(nb + 1) * P, :], in_=t32s[:])

            # ---------------- P1a: gate + logits ----------------
            for db in range(NDB):
                gs = gath.tile([P, CPD * 32], F32, tag="t32gs")
                gd = gath.tile([P, CPD * 32], F32, tag="t32gd")
                for k in range(CPD):
                    c = db * CPD + k
                    nc.gpsimd.indirect_dma_start(
                        out=gs[:, k * 32:(k + 1) * 32], out_offset=None,
                        in_=t32d.ap(), in_offset=IOX(srcidx_t[:, c:c + 1]))
                    nc.gpsimd.indirect_dma_start(
                        out=gd[:, k * 32:(k + 1) * 32], out_offset=None,
                        in_=t32d.ap(), in_offset=IOX(dstidx_t[:, c:c + 1]))
                gsv = bass.AP(gs[:].tensor, gs[:].offset,
                              [gs[:].ap[0], [32, CPD], [1, 8]])
                gdv = bass.AP(gd[:].tensor, gd[:].offset + 16,
                              [gd[:].ap[0], [32, CPD], [1, 8]])
                tt = work.tile([P, CPD * 8], F32, tag="tt")
                ttv = bass.AP(tt[:].tensor, tt[:].offset, [tt[:].ap[0], [8, CPD], [1, 8]])
                nc.vector.tensor_tensor(out=ttv, in0=gsv, in1=gdv,
                                        op=mybir.AluOpType.add)
                c2v = bass.AP(c2_t[:].tensor, c2_t[:].offset,
                              [c2_t[:].ap[0], [0, CPD], [1, 8]])
                nc.vector.tensor_tensor(out=ttv, in0=ttv, in1=c2v,
                                        op=mybir.AluOpType.add)
                nc.vector.tensor_scalar_max(tt[:], tt[:], 0.0)
                w3v = bass.AP(w3_t[:].tensor, w3_t[:].offset,
                              [w3_t[:].ap[0], [0, CPD], [1, 8]])
                nc.vector.tensor_tensor(out=ttv, in0=ttv, in1=w3v,
                                        op=mybir.AluOpType.mult)
                sv = bass.AP(s_sl[:].tensor, s_sl[:].offset + db * CPD,
                             [s_sl[:].ap[0], [1, CPD], [1, 1]])
                nc.vector.tensor_reduce(out=sv, in_=ttv, op=mybir.AluOpType.add, axis=mybir.AxisListType.X)
                # attention logits
                gsv2 = bass.AP(gs[:].tensor, gs[:].offset + 8,
                               [gs[:].ap[0], [32, CPD], [1, 8]])
                gdv2 = bass.AP(gd[:].tensor, gd[:].offset + 24,
                               [gd[:].ap[0], [32, CPD], [1, 8]])
                elv = bass.AP(el_sl[:].tensor, el_sl[:].offset + db * CPD * 8,
                              [el_sl[:].ap[0], [8, CPD], [1, 8]])
                nc.vector.tensor_tensor(out=elv, in0=gsv2, in1=gdv2,
                                        op=mybir.AluOpType.add)
                nc.scalar.activation(out=elv, in_=elv,
                                     func=mybir.ActivationFunctionType.Lrelu,
                                     alpha=0.01)

            # ---------------- P1b: minmax -> gate -> exp ----------------
            invb_t = work.tile([P, NCHUNK], F32, tag="invb")
            nc.sync.dma_start(out=invb_t[:], in_=invbias.ap())
            smin_in = work.tile([P, NCHUNK], F32, tag="smm")
            nc.vector.tensor_tensor(out=smin_in[:], in0=s_sl[:], in1=invb_t[:],
                                    op=mybir.AluOpType.add)
            packed = work.tile([P, 2], F32, tag="packed")
            nc.vector.tensor_reduce(out=packed[:, 0:1], in_=smin_in[:],
                                    op=mybir.AluOpType.min, axis=mybir.AxisListType.X)
            nc.vector.tensor_tensor(out=smin_in[:], in0=s_sl[:], in1=invb_t[:],
                                    op=mybir.AluOpType.subtract)
            nc.vector.tensor_reduce(out=packed[:, 1:2], in_=smin_in[:],
                                    op=mybir.AluOpType.max, axis=mybir.AxisListType.X)
            # negate min so cross-partition/core reduce is a single max
            nc.vector.tensor_scalar_mul(packed[:, 0:1], packed[:, 0:1], -1.0)
            ptp = psum.tile([2, P], F32, tag="tp", space="PSUM")
            nc.tensor.transpose(out=ptp[:], in_=packed[:], identity=idn_t[:])
            mm2 = work.tile([2, 1], F32, tag="mm2")
            nc.vector.tensor_reduce(out=mm2[:], in_=ptp[:], op=mybir.AluOpType.max, axis=mybir.AxisListType.X)
            nc.sync.dma_start(out=mm_in.ap(), in_=mm2[:])
            nc.gpsimd.collective_compute(
                "AllReduce", mybir.AluOpType.max, replica_groups=RG,
                ins=[mm_in.ap()], outs=[mm_out.ap()])
            mmr = work.tile([1, 2], F32, tag="mmr")
            nc.sync.dma_start(out=mmr[:], in_=mm_out.ap())
            # broadcast to all partitions via matmul with ones column
            ones1 = slab.tile([1, P], F32)
            nc.gpsimd.memset(ones1[:], 1.0)
            pbc = psum.tile([P, 2], F32, tag="tp", space="PSUM")
            nc.tensor.matmul(out=pbc[:], lhsT=ones1[:], rhs=mmr[:],
                             start=True, stop=True)
            mmb = work.tile([P, 2], F32, tag="mmb")
            nc.vector.tensor_copy(out=mmb[:], in_=pbc[:])  # [:,0]=-min [:,1]=max
            rng_t = work.tile([P, 1], F32, tag="rngt")
            nc.vector.tensor_tensor(out=rng_t[:], in0=mmb[:, 1:2], in1=mmb[:, 0:1],
                                    op=mybir.AluOpType.add)  # max-min
            inv_t = work.tile([P, 1], F32, tag="invt")
            nc.vector.reciprocal(out=inv_t[:], in_=rng_t[:])
            nc.vector.tensor_scalar_add(gate_sl[:], s_sl[:], mmb[:, 0:1])
            nc.vector.tensor_scalar_mul(gate_sl[:], gate_sl[:], inv_t[:, 0:1])
            # eg = el * gate ; ex = exp(eg)
            elv_all = bass.AP(el_sl[:].tensor, el_sl[:].offset,
                              [el_sl[:].ap[0], [8, NCHUNK], [1, 8]])
            gatev = bass.AP(gate_sl[:].tensor, gate_sl[:].offset,
                            [gate_sl[:].ap[0], [1, NCHUNK], [0, 8]])
            exv_all = bass.AP(ex_sl[:].tensor, ex_sl[:].offset,
                              [ex_sl[:].ap[0], [8, NCHUNK], [1, 8]])
            nc.vector.tensor_tensor(out=exv_all, in0=elv_all, in1=gatev,
                                    op=mybir.AluOpType.mult)
            nc.scalar.activation(out=ex_sl[:], in_=ex_sl[:],
                                 func=mybir.ActivationFunctionType.Exp)

            # ---------------- P1c: layer-1 aggregation ----------------
            for db in range(NDB):
                U = psum.tile([P, 264], F32, tag="acc", space="PSUM")
                CPDH = (CPD + 1) // 2
                z1gs = []
                for g in range(2):
                    k0, k1 = g * CPDH, min((g + 1) * CPDH, CPD)
                    if k0 >= k1:
                        break
                    z1g = gath.tile([P, CPDH * 256], F32, tag="z1g")
                    for k in range(k0, k1):
                        c = db * CPD + k
                        nc.gpsimd.indirect_dma_start(
                            out=z1g[:, (k - k0) * 256:(k - k0 + 1) * 256],
                            out_offset=None,
                            in_=z1d.ap(), in_offset=IOX(srcidx_t[:, c:c + 1]))
                    z1gs.append((k0, z1g))
                for k in range(CPD):
                    c = db * CPD + k
                    k0, z1g = z1gs[0] if k < CPDH else z1gs[1]
                    stg = work.tile([P, 264], F32, tag="stg")
                    exs = bass.AP(ex_sl[:].tensor, ex_sl[:].offset + c * 8,
                                  [ex_sl[:].ap[0], [1, 8], [0, 32]])
                    stgv = bass.AP(stg[:].tensor, stg[:].offset,
                                   [stg[:].ap[0], [32, 8], [1, 32]])
                    z1gv = bass.AP(z1g[:].tensor, z1g[:].offset + (k - k0) * 256,
                                   [z1g[:].ap[0], [32, 8], [1, 32]])
                    nc.vector.tensor_tensor(out=stgv, in0=z1gv, in1=exs,
                                            op=mybir.AluOpType.mult)
                    nc.vector.tensor_copy(out=stg[:, 256:264],
                                          in_=ex_sl[:, c * 8:(c + 1) * 8])
                    oh = work.tile([P, P], F32, tag="oh")
                    dlb = dstloc_t[:, c:c + 1].to_broadcast([P, P])
                    nc.vector.tensor_tensor(out=oh[:], in0=dlb, in1=iota_t[:],
                                            op=mybir.AluOpType.is_equal)
                    nc.tensor.matmul(out=U[:], lhsT=oh[:], rhs=stg[:],
                                     start=(k == 0), stop=(k == CPD - 1))
                den = work.tile([P, 8], F32, tag="den")
                nc.vector.tensor_scalar_max(den[:], U[:, 256:264], 1e-12)
                rden = work.tile([P, 8], F32, tag="rden")
                nc.vector.reciprocal(out=rden[:], in_=den[:])
                hAv = bass.AP(hA_sl[:].tensor, hA_sl[:].offset + db * 256,
                              [hA_sl[:].ap[0], [32, 8], [1, 32]])
                Uv = bass.AP(U[:].tensor, U[:].offset, [U[:].ap[0], [32, 8], [1, 32]])
                rdv = bass.AP(rden[:].tensor, rden[:].offset,
                              [rden[:].ap[0], [1, 8], [0, 32]])
                nc.vector.tensor_tensor(out=hAv, in0=Uv, in1=rdv,
                                        op=mybir.AluOpType.mult)
                nc.scalar.activation(out=hA_sl[:, db * 256:(db + 1) * 256],
                                     in_=hA_sl[:, db * 256:(db + 1) * 256],
                                     func=mybir.ActivationFunctionType.Lrelu,
                                     alpha=0.01)

            # ---------------- P2: z2/es2/ed2 + AllGather ----------------
            for db in range(NDB):
                pz = psum.tile([P, 66], F32, tag="acc", space="PSUM")
                for half in range(2):
                    pt = psum.tile([P, P], F32, tag="tp", space="PSUM")
                    nc.tensor.transpose(
                        out=pt[:],
                        in_=hA_sl[:, db * 256 + half * 128: db * 256 + half * 128 + 128],
                        identity=idn_t[:])
                    hAT = work.tile([P, P], F32, tag="hAT")
                    nc.vector.tensor_copy(out=hAT[:], in_=pt[:])
                    nc.tensor.matmul(out=pz[:], lhsT=hAT[:],
                                     rhs=fc2_t[:, half * 66:(half + 1) * 66],
                                     start=(half == 0), stop=(half == 1))
                z2s = work.tile([P, 66], F32, tag="z2s")
                nc.vector.tensor_copy(out=z2s[:], in_=pz[:])
                nc.sync.dma_start(out=z2in.ap()[db * P:(db + 1) * P, :], in_=z2s[:])
            nc.gpsimd.collective_compute(
                "AllGather", mybir.AluOpType.bypass, replica_groups=RG,
                ins=[z2in.ap()], outs=[z2all.ap()])

            # ---------------- P3: layer-2 aggregation ----------------
            for db in range(NDB):
                z2g = gath.tile([P, CPD * 66], F32, tag="z2g")
                z2gd = gath.tile([P, CPD * 66], F32, tag="z2gd")
                for k in range(CPD):
                    c = db * CPD + k
                    nc.gpsimd.indirect_dma_start(
                        out=z2g[:, k * 66:(k + 1) * 66], out_offset=None,
                        in_=z2all.ap(), in_offset=IOX(srcidx_t[:, c:c + 1]))
                    nc.gpsimd.indirect_dma_start(
                        out=z2gd[:, k * 66:(k + 1) * 66], out_offset=None,
                        in_=z2all.ap(), in_offset=IOX(dstidx_t[:, c:c + 1]))
                e2 = work.tile([P, CPD], F32, tag="e2")
                es2v = bass.AP(z2g[:].tensor, z2g[:].offset + 64,
                               [z2g[:].ap[0], [66, CPD], [1, 1]])
                ed2v = bass.AP(z2gd[:].tensor, z2gd[:].offset + 65,
                               [z2gd[:].ap[0], [66, CPD], [1, 1]])
                e2v = bass.AP(e2[:].tensor, e2[:].offset, [e2[:].ap[0], [1, CPD], [1, 1]])
                nc.vector.tensor_tensor(out=e2v, in0=es2v, in1=ed2v,
                                        op=mybir.AluOpType.add)
                nc.scalar.activation(out=e2[:], in_=e2[:],
                                     func=mybir.ActivationFunctionType.Lrelu,
                                     alpha=0.01)
                nc.vector.tensor_tensor(out=e2[:], in0=e2[:],
                                        in1=gate_sl[:, db * CPD:(db + 1) * CPD],
                                        op=mybir.AluOpType.mult)
                nc.scalar.activation(out=e2[:], in_=e2[:],
                                     func=mybir.ActivationFunctionType.Exp)
                U2 = psum.tile([P, 65], F32, tag="acc", space="PSUM")
                for k in range(CPD):
                    c = db * CPD + k
                    stg2 = work.tile([P, 65], F32, tag="stg2")
                    ex2b = e2[:, k:k + 1].to_broadcast([P, 64])
                    nc.vector.tensor_tensor(out=stg2[:, 0:64],
                                            in0=z2g[:, k * 66:k * 66 + 64],
                                            in1=ex2b, op=mybir.AluOpType.mult)
                    nc.vector.tensor_copy(out=stg2[:, 64:65], in_=e2[:, k:k + 1])
                    oh = work.tile([P, P], F32, tag="oh")
                    dlb = dstloc_t[:, c:c + 1].to_broadcast([P, P])
                    nc.vector.tensor_tensor(out=oh[:], in0=dlb, in1=iota_t[:],
                                            op=mybir.AluOpType.is_equal)
                    nc.tensor.matmul(out=U2[:], lhsT=oh[:], rhs=stg2[:],
                                     start=(k == 0), stop=(k == CPD - 1))
                den2 = work.tile([P, 1], F32, tag="den2")
                nc.vector.tensor_scalar_max(den2[:], U2[:, 64:65], 1e-12)
                rden2 = work.tile([P, 1], F32, tag="rden2")
                nc.vector.reciprocal(out=rden2[:], in_=den2[:])
                rd2b = rden2[:, 0:1].to_broadcast([P, 64])
                nc.vector.tensor_tensor(out=hB_sl[:, db * 64:(db + 1) * 64],
                                        in0=U2[:, 0:64], in1=rd2b,
                                        op=mybir.AluOpType.mult)

            # ---------------- P4: q/r + AllGather ----------------
            for db in range(NDB):
                pt4 = psum.tile([64, P], F32, tag="tp", space="PSUM")
                nc.tensor.transpose(out=pt4[:],
                                    in_=hB_sl[:, db * 64:(db + 1) * 64],
                                    identity=idn_t[:])
                hBT = work.tile([64, P], F32, tag="hBT")
                nc.vector.tensor_copy(out=hBT[:], in_=pt4[:])
                pq = psum.tile([P, 4], F32, tag="acc", space="PSUM")
                nc.tensor.matmul(out=pq[:], lhsT=hBT[:], rhs=wqr_t[:],
                                 start=True, stop=True)
                qr = work.tile([P, 4], F32, tag="qr")
                nc.vector.tensor_copy(out=qr[:, 0:2], in_=pq[:, 0:2])
                nc.vector.tensor_tensor(out=qr[:, 2:4], in0=pq[:, 2:4],
                                        in1=bp_t[:], op=mybir.AluOpType.add)
                nc.sync.dma_start(out=qrin.ap()[db * P:(db + 1) * P, :], in_=qr[:])
            nc.gpsimd.collective_compute(
                "AllGather", mybir.AluOpType.bypass, replica_groups=RG,
                ins=[qrin.ap()], outs=[qrall.ap()])

            # ---------------- P5: edge scores ----------------
            for db in range(NDB):
                qs = gath.tile([P, CPD * 4], F32, tag="qs")
                qd = gath.tile([P, CPD * 4], F32, tag="qd")
                for k in range(CPD):
                    c = db * CPD + k
                    nc.gpsimd.indirect_dma_start(
                        out=qs[:, k * 4:(k + 1) * 4], out_offset=None,
                        in_=qrall.ap(), in_offset=IOX(srcidx_t[:, c:c + 1]))
                    nc.gpsimd.indirect_dma_start(
                        out=qd[:, k * 4:(k + 1) * 4], out_offset=None,
                        in_=qrall.ap(), in_offset=IOX(dstidx_t[:, c:c + 1]))
                qsv = bass.AP(qs[:].tensor, qs[:].offset,
                              [qs[:].ap[0], [4, CPD], [1, 2]])
                qdv = bass.AP(qd[:].tensor, qd[:].offset + 2,
                              [qd[:].ap[0], [4, CPD], [1, 2]])
                scv = bass.AP(score_sl[:].tensor, score_sl[:].offset + db * CPD * 2,
                              [score_sl[:].ap[0], [2, CPD], [1, 2]])
                nc.vector.tensor_tensor(out=scv, in0=qsv, in1=qdv,
                                        op=mybir.AluOpType.add)
            nc.sync.dma_start(out=score_out.ap(), in_=score_sl[:])
            nc.sync.dma_start(out=gate_out.ap(), in_=gate_sl[:])

    nc.compile()
    _cache[key] = nc
    return nc


def kernel(h, src, dst, W1, b1, W2, b2, W3, b3,
           fc1, a1_src, a1_dst, fc2, a2_src, a2_dst, Wp, bp, _trace=False):
    h = np.asarray(h, np.float32)
    src = np.asarray(src, np.int64)
    dst = np.asarray(dst, np.int64)
    N, IN = h.shape
    E = src.shape[0]
    HEADS = np.asarray(fc1).shape[0]
    HID = np.asarray(fc1).shape[2]
    OUT = np.asarray(fc2).shape[2]
    assert IN == 128 and HEADS == 8 and HID == 32 and OUT == 64

    NPC = ((N + NCORES * P - 1) // (NCORES * P)) * P
    NP = NPC * NCORES
    NDB = NPC // P

    core = dst // NPC
    dbl = (dst % NPC) // P
    # counts per (core, db)
    cnt = np.zeros((NCORES, NDB), np.int64)
    np.add.at(cnt, (core, dbl), 1)
    CPD = int((cnt.max() + P - 1) // P)
    NCHUNK = NDB * CPD

    order = np.lexsort((np.arange(E), dbl, core))
    # per (core,db) slot assignment
    srcg = np.zeros((NCORES, NCHUNK * P), np.int32)
    dstg = np.zeros((NCORES, NCHUNK * P), np.int32)
    dlocg = np.full((NCORES, NCHUNK * P), -1.0, np.float32)
    invbg = np.full((NCORES, NCHUNK * P), 1e30, np.float32)
    eidg = np.full((NCORES, NCHUNK * P), -1, np.int64)
    oc = core[order]
    od = dbl[order]
    pos_in_grp = np.zeros(E, np.int64)
    # compute position within each (core,db) group via cumcount on sorted order
    grp = oc * NDB + od
    changes = np.r_[True, grp[1:] != grp[:-1]]
    gstart = np.where(changes)[0]
    gid = np.cumsum(changes) - 1
    pos_in_grp = np.arange(E) - gstart[gid]
    slot = od * (CPD * P) + pos_in_grp  # within-core flat slot
    srcg[oc, slot] = src[order].astype(np.int32)
    dstg[oc, slot] = dst[order].astype(np.int32)
    dlocg[oc, slot] = (dst[order] % P).astype(np.float32)
    invbg[oc, slot] = 0.0
    eidg[oc, slot] = order

    def wrap(a):
        # [NCHUNK*P] -> [P, NCHUNK] with edge j=(chunk c, part p) at [p, c]
        return np.ascontiguousarray(a.reshape(NCHUNK, P).T)

    # host-folded weights
    W1f = np.asarray(W1, np.float32); b1f = np.asarray(b1, np.float32)
    W2f = np.asarray(W2, np.float32); b2f = np.asarray(b2, np.float32)
    W3f = np.asarray(W3, np.float32)
    fc1f = np.asarray(fc1, np.float32)
    a1s = np.asarray(a1_src, np.float32); a1d = np.asarray(a1_dst, np.float32)
    fc2f = np.asarray(fc2, np.float32)[0]
    a2s = np.asarray(a2_src, np.float32)[0]; a2d = np.asarray(a2_dst, np.float32)[0]
    Wpf = np.asarray(Wp, np.float32); bpf = np.asarray(bp, np.float32)

    fc1_flat = fc1f.transpose(1, 0, 2).reshape(IN, HEADS * HID)
    w_u = W1f @ W2f[:16]           # [128,8]
    w_v = W1f @ W2f[16:]           # [128,8]
    w_es = np.stack([fc1f[hh] @ a1s[hh] for hh in range(HEADS)], 1)  # [128,8]
    w_ed = np.stack([fc1f[hh] @ a1d[hh] for hh in range(HEADS)], 1)
    Wall = np.concatenate([fc1_flat, w_u, w_es, w_v, w_ed], 1)  # [128,288]
    c2 = b1f @ W2f[:16] + b1f @ W2f[16:] + b2f  # [8]
    fc2aug = np.concatenate([fc2f, (fc2f @ a2s)[:, None],
                             (fc2f @ a2d)[:, None]], 1)  # [256,66]
    Wqr = np.concatenate([Wpf[:OUT], Wpf[OUT:]], 1)  # [64,4]

    hT = np.zeros((P, NP), np.float32)
    hT[:, :N] = h.T
    consts = {
        "hT": hT,
        "Wall": Wall,
        "c2row": np.tile(c2[None, :], (P, 1)).astype(np.float32),
        "W3row": np.tile(W3f[:, 0][None, :], (P, 1)).astype(np.float32),
        "fc2aug": np.ascontiguousarray(fc2aug.reshape(2, P, 66).transpose(1, 0, 2)).reshape(P, 132).astype(np.float32),
        "Wqr": Wqr.astype(np.float32),
        "bprow": np.tile(bpf[None, :], (P, 1)).astype(np.float32),
        "IOTA": np.tile(np.arange(P, dtype=np.float32)[None, :], (P, 1)),
        "IDN": np.eye(P, dtype=np.float32),
    }
    in_maps = []
    for r in range(NCORES):
        m = dict(consts)
        m["srcidx"] = wrap(srcg[r])
        m["dstidx"] = wrap(dstg[r])
        m["dstloc"] = wrap(dlocg[r])
        m["invbias"] = wrap(invbg[r])
        in_maps.append(m)

    nc = _build(NP, NPC, NDB, CPD)
    try:
        res = run_bass_kernel_spmd(nc, in_maps, list(range(NCORES)),
                                   trace=_trace)
    except ModuleNotFoundError:
        res = run_bass_kernel_spmd(nc, in_maps, list(range(NCORES)))
    if _trace and getattr(res, "exec_time_ns", None):
        print("HW exec time:", res.exec_time_ns, "ns")

    edge_score = np.zeros((E, 2), np.float32)
    gate = np.zeros((E, 1), np.float32)
    for r in range(NCORES):
        sc = res.results[r]["score_out"].reshape(P, NCHUNK, 2)
        gt = res.results[r]["gate_out"].reshape(P, NCHUNK)
        eid = eidg[r].reshape(NCHUNK, P).T
        valid = eid >= 0
        edge_score[eid[valid]] = sc[valid]
        gate[eid[valid], 0] = gt[valid]
    return edge_score, gate
